# revision 1
# baseline (speedup 1.0000x reference)
"""Multi-head attention (B=1, S=2048, H=1024, NH=16) on 8 trn2 NeuronCores.

Sharding: head-parallel. Core c owns heads {2c, 2c+1} (= 128 of the 1024
hidden dims). Each core computes its Q/K/V projection slices, the full
attention for its 2 heads, and a full-width partial of the output
projection (contraction over its 128 context dims). Host sums the 8
partials and adds the (host-folded) biases.

Device layouts (per core):
  qT/kT     [128 n, 2048 t]  n = 2 heads x 64 dims on partitions
  S.T tiles [128 j, i]       mask-multiply fused into the PSUM eviction
  O         [128 i, 65]      PV matmul with E' stationary; col 64 is the
                             softmax denominator (ones column in v) ->
                             per-partition normalize
  y partial [1024 n, 2048 t] bf16, host-transposed/summed in fp32

Precision: all matmuls bf16 with fp32 PSUM accumulation. The 0/1 mask is
stored fp8-e4m3 (exact, halves its bandwidth). Softmax runs without
max-subtraction: the exponent is (q.k/8)*M ~ N(0, 0.33^2), so exp never
overflows.
"""

import math

import numpy as np
import ml_dtypes

BF16 = ml_dtypes.bfloat16
FP8 = ml_dtypes.float8_e4m3
S, H, NH, DK = 2048, 1024, 16, 64
NCORES = 8
HPC = NH // NCORES          # heads per core = 2
DPC = HPC * DK              # head dims per core = 128
KC = H // 128               # contraction chunks = 8
TP = S // 512               # 512-wide token panels = 4
JC = S // 128               # 128-wide key chunks = 16
VA = DK + 1                 # v columns + ones column = 65

_CACHE = {}


def _oslc(ic):
    """o_ps column offset for ic-th 65-wide slice: 7 slices per 512-fp32
    PSUM bank so no matmul crosses a bank boundary."""
    b, r = divmod(ic, 7)
    return b * 512 + r * VA


def _build_program():
    """Build + compile the (identical) per-core Bass program."""
    from contextlib import ExitStack

    import concourse.bacc as bacc
    import concourse.tile as tile
    from concourse import mybir

    dt = mybir.dt
    AF = mybir.ActivationFunctionType
    f8 = dt.float8e4

    nc = bacc.Bacc("TRN2", target_bir_lowering=False, debug=False)

    qT_d = nc.dram_tensor("qT", [H, S], dt.bfloat16, kind="ExternalInput").ap()
    kT_d = nc.dram_tensor("kT", [H, S], dt.bfloat16, kind="ExternalInput").ap()
    vT_d = nc.dram_tensor("vT", [H, S], dt.bfloat16, kind="ExternalInput").ap()
    maskT_d = nc.dram_tensor("maskT", [S, S], f8, kind="ExternalInput").ap()
    wq_d = nc.dram_tensor("wq", [128, KC * DPC], dt.bfloat16, kind="ExternalInput").ap()
    wk_d = nc.dram_tensor("wk", [128, KC * DPC], dt.bfloat16, kind="ExternalInput").ap()
    wv_d = nc.dram_tensor("wv", [128, KC * DPC], dt.bfloat16, kind="ExternalInput").ap()
    wo_d = nc.dram_tensor("wo", [DPC, H], dt.bfloat16, kind="ExternalInput").ap()
    bq_d = nc.dram_tensor("bq", [1, DPC], dt.bfloat16, kind="ExternalInput").ap()
    bk_d = nc.dram_tensor("bk", [1, DPC], dt.bfloat16, kind="ExternalInput").ap()
    id_d = nc.dram_tensor("ident", [128, 128], dt.bfloat16, kind="ExternalInput").ap()
    yT_d = nc.dram_tensor("yT", [H, S], dt.bfloat16, kind="ExternalOutput").ap()

    with tile.TileContext(nc) as tc, ExitStack() as ctx:
        cp = ctx.enter_context(tc.tile_pool(name="const", bufs=1))
        sm_p = ctx.enter_context(tc.tile_pool(name="sm", bufs=3))
        e_p = ctx.enter_context(tc.tile_pool(name="ex", bufs=3))
        ot_p = ctx.enter_context(tc.tile_pool(name="otok", bufs=3))
        rc_p = ctx.enter_context(tc.tile_pool(name="recip", bufs=3))
        y_p = ctx.enter_context(tc.tile_pool(name="ysb", bufs=2))

        # ---- small constants + weights first (unblock PE asap) ----
        ones_row = cp.tile([1, 512], dt.bfloat16, tag="ones")
        nc.vector.memset(ones_row, 1.0)
        w_sb = {}
        for name, d in (("wq", wq_d), ("wk", wk_d), ("wv", wv_d)):
            w = cp.tile([128, KC * DPC], dt.bfloat16, tag=name, name=name)
            nc.sync.dma_start(out=w, in_=d)
            w_sb[name] = w
        wo_sb = cp.tile([128, H], dt.bfloat16, tag="wo")
        nc.sync.dma_start(out=wo_sb, in_=wo_d)
        bq_sb = cp.tile([1, DPC], dt.bfloat16, tag="bq")
        nc.sync.dma_start(out=bq_sb, in_=bq_d)
        bk_sb = cp.tile([1, DPC], dt.bfloat16, tag="bk")
        nc.sync.dma_start(out=bk_sb, in_=bk_d)

        qT_sb = cp.tile([128, S], dt.bfloat16, tag="qTs")
        kT_sb = cp.tile([128, S], dt.bfloat16, tag="kTs")
        vaug = cp.tile([128, JC * (HPC * VA)], dt.bfloat16, tag="vaug")
        ident = cp.tile([128, 128], dt.bfloat16, tag="ident")
        nc.sync.dma_start(out=ident, in_=id_d)
        oT_sb = [cp.tile([128, 512], dt.bfloat16, tag=f"oTp{p}", name=f"oTp{p}")
                 for p in range(TP)]
        y_sb = [cp.tile([128, S], dt.bfloat16, tag=f"ysb{nn}", name=f"ysb{nn}")
                for nn in range(KC)]
        mask_sb = cp.tile([128, JC * S], f8, tag="mask")

        # ---- Q+K projections; inputs in 1MB DMAs (4 chunks each) ----
        with tc.tile_pool(name="ps_proj", bufs=1, space="PSUM") as pq, \
             tc.tile_pool(name="xin", bufs=1) as xin_p:
            projs = (("wq", qT_d, bq_sb, qT_sb, "q"), ("wk", kT_d, bk_sb, kT_sb, "k"))
            psl = {}
            xin = {}
            for name, x_d, b_sb, dest, pre in projs:
                psl[pre] = [
                    pq.tile([128, 512], dt.float32, tag=f"p{pre}{p}", name=f"p{pre}{p}")
                    for p in range(TP)
                ]
            # DMA halves in arrival order q0,k0,q1,k1; MMs follow the data
            for hf in range(2):
                for name, x_d, b_sb, dest, pre in projs:
                    xt = xin_p.tile(
                        [128, 4 * S], dt.bfloat16, tag=f"x{pre}{hf}", name=f"x{pre}{hf}"
                    )
                    nc.sync.dma_start(
                        out=xt.rearrange("p (c i) -> p c i", c=4),
                        in_=x_d[hf * 512 : (hf + 1) * 512, :].rearrange(
                            "(c p) i -> p c i", p=128
                        ),
                    )
                    xin[pre, hf] = xt
            def proj_mms(pre, name, hf):
                for c in range(4):
                    kk = hf * 4 + c
                    for p in range(TP):
                        nc.tensor.matmul(
                            psl[pre][p],
                            lhsT=w_sb[name][:, kk * DPC : (kk + 1) * DPC],
                            rhs=xin[pre, hf][:, c * S + p * 512 : c * S + (p + 1) * 512],
                            start=(kk == 0),
                            stop=False,
                        )

            def proj_finish(pre, b_sb, dest, p):
                nc.tensor.matmul(
                    psl[pre][p], lhsT=b_sb, rhs=ones_row, start=False, stop=True
                )
                if p % 2 == 0:
                    nc.scalar.activation(
                        dest[:, p * 512 : (p + 1) * 512], psl[pre][p], AF.Copy
                    )
                else:
                    nc.vector.tensor_copy(
                        dest[:, p * 512 : (p + 1) * 512], psl[pre][p]
                    )

            proj_mms("q", "wq", 0)
            proj_mms("k", "wk", 0)
            proj_mms("q", "wq", 1)
            for p in range(TP):
                proj_finish("q", bq_sb, qT_sb, p)
            proj_mms("k", "wk", 1)
            for p in range(TP):
                proj_finish("k", bk_sb, kT_sb, p)

        # V inputs as 16 half-tiles (first half usable before second lands),
        # then the mask in 4 batched DMAs (consumed later than v)
        with tc.tile_pool(name="vin", bufs=1) as vin_p:
            vin = [[None, None] for _ in range(KC)]

            def mask_group(g):
                nc.sync.dma_start(
                    out=mask_sb[:, g * 4 * S : (g + 1) * 4 * S].rearrange(
                        "p (a i) -> p a i", a=4
                    ),
                    in_=maskT_d[g * 512 : (g + 1) * 512, :].rearrange(
                        "(a p) i -> p a i", p=128
                    ),
                )

            def vin_half(th):
                for kk in range(KC):
                    t_ = vin_p.tile(
                        [128, S // 2], dt.bfloat16,
                        tag=f"vin{kk}_{th}", name=f"vin{kk}_{th}",
                    )
                    nc.sync.dma_start(
                        out=t_,
                        in_=vT_d[kk * 128 : (kk + 1) * 128,
                                 th * (S // 2) : (th + 1) * (S // 2)],
                    )
                    vin[kk][th] = t_

            mask_group(0)
            vin_half(0)
            mask_group(1)
            vin_half(1)
            mask_group(2)
            mask_group(3)

            # ---- attention; V projection interleaved into the h=0 j-loop ----
            # PSUM: misc(pv+tp shared tag) 1 bank + s 2x2 + o 3 = 8 banks
            with tc.tile_pool(name="ps_misc", bufs=1, space="PSUM") as pm, \
                 tc.tile_pool(name="ps_s", bufs=2, space="PSUM") as ps_p, \
                 tc.tile_pool(name="ps_o", bufs=1, space="PSUM") as po_p:

                def v_proj_chunk(t):
                    """Token-chunk t of the V projection into vaug."""
                    ps = pm.tile([128, DPC], dt.float32, tag="misc", name=f"pv{t}")
                    th, ts_ = divmod(t, 8)
                    for kk in range(KC):
                        nc.tensor.matmul(
                            ps,
                            lhsT=vin[kk][th][:, ts_ * 128 : (ts_ + 1) * 128],
                            rhs=w_sb["wv"][:, kk * DPC : (kk + 1) * DPC],
                            start=(kk == 0),
                            stop=(kk == KC - 1),
                        )
                    base = t * (HPC * VA)
                    for h in range(HPC):
                        nc.scalar.activation(
                            vaug[:, base + h * VA : base + h * VA + DK],
                            ps[:, h * DK : (h + 1) * DK],
                            AF.Copy,
                        )
                        nc.vector.memset(
                            vaug[:, base + h * VA + DK : base + h * VA + VA], 1.0
                        )

                def pv_mms(h, j, et, o_ps):
                    for ic in range(JC):
                        nc.tensor.matmul(
                            o_ps[:, _oslc(ic) : _oslc(ic) + VA],
                            lhsT=et[:, ic * 128 : (ic + 1) * 128],
                            rhs=vaug[:, j * (HPC * VA) + h * VA : j * (HPC * VA) + (h + 1) * VA],
                            start=(j == 0 and ic % 7 == 0),
                            stop=(j == JC - 1 and (ic % 7 == 6 or ic == JC - 1)),
                        )

                for h in range(HPC):
                    hs = h * DK
                    o_ps = po_p.tile([128, 1536], dt.float32, tag="ops")
                    pend = None  # (j, et) whose PV matmuls are not yet emitted
                    for j in range(JC):
                        sm = sm_p.tile([128, S], dt.bfloat16, tag="sm")
                        for half in range(2):
                            s_ps = ps_p.tile([128, 1024], dt.float32, tag="sps")
                            for q in range(2):
                                pi = half * 2 + q
                                nc.tensor.matmul(
                                    s_ps[:, q * 512 : (q + 1) * 512],
                                    lhsT=kT_sb[hs : hs + DK, j * 128 : (j + 1) * 128],
                                    rhs=qT_sb[hs : hs + DK, pi * 512 : (pi + 1) * 512],
                                    start=True,
                                    stop=True,
                                )
                            nc.vector.tensor_mul(
                                sm[:, half * 1024 : (half + 1) * 1024],
                                s_ps,
                                mask_sb[:, j * S + half * 1024 : j * S + (half + 1) * 1024],
                            )
                        et = e_p.tile([128, S], dt.bfloat16, tag="et")
                        nc.scalar.activation(et, sm, AF.Exp, scale=1.0 / math.sqrt(DK))
                        # software pipeline: PE emits S(j+1) before PV(j), so
                        # the S->TT->exp->PV chain doesn't serialize per j.
                        # The V projection chunk rides the same pipeline slot.
                        if pend is not None:
                            if h == 0:
                                v_proj_chunk(pend[0])
                            pv_mms(h, pend[0], pend[1], o_ps)
                        pend = (j, et)
                    if h == 0:
                        v_proj_chunk(pend[0])
                    pv_mms(h, pend[0], pend[1], o_ps)
                    # epilogue: per PSUM bank (7 ic-slices), batch-reciprocal
                    # the denominator columns and batch-normalize via a 3D
                    # strided AP with the recip broadcast (step-0) over DK
                    import concourse.bass as bass_mod
                    ot_big = ot_p.tile([128, JC * DK], dt.bfloat16, tag="ot")
                    for b in range(3):
                        n_ic = (7, 7, 2)[b]
                        rc = rc_p.tile([128, 8], dt.float32, tag="rc", name=f"rc{h}_{b}")
                        den = bass_mod.AP(
                            tensor=o_ps.tensor,
                            offset=o_ps.offset + b * 512 + DK,
                            ap=[o_ps.ap[0], [VA, n_ic]],
                        )
                        nc.vector.reciprocal(rc[:, :n_ic], den)
                        src_ap = bass_mod.AP(
                            tensor=o_ps.tensor,
                            offset=o_ps.offset + b * 512,
                            ap=[o_ps.ap[0], [VA, n_ic], [1, DK]],
                        )
                        rcb = bass_mod.AP(
                            tensor=rc.tensor,
                            offset=rc.offset,
                            ap=[rc.ap[0], [1, n_ic], [0, DK]],
                        )
                        dst = ot_big[:, b * 7 * DK : (b * 7 + n_ic) * DK].rearrange(
                            "p (a d) -> p a d", d=DK
                        )
                        nc.vector.tensor_mul(dst, src_ap, rcb)
                    for ic in range(JC):
                        ot = ot_big[:, ic * DK : (ic + 1) * DK]
                        if h == HPC - 1 and ic % 2 == 0:
                            tp = ps_p.tile([DK, 128], dt.bfloat16, tag="sps", name=f"tp{h}_{ic}")
                        else:
                            tp = pm.tile([DK, 128], dt.bfloat16, tag="misc", name=f"tp{h}_{ic}")
                        nc.tensor.transpose(tp, ot, ident)
                        nc.vector.tensor_copy(
                            oT_sb[ic // 4][hs : hs + DK, (ic % 4) * 128 : (ic % 4 + 1) * 128],
                            tp,
                        )
                        if h == HPC - 1 and ic % 4 == 3:
                            p = ic // 4
                            for nn in range(KC):
                                y_ps = ps_p.tile(
                                    [128, 512], dt.float32, tag="sps", name=f"y{p}_{nn}"
                                )
                                nc.tensor.matmul(
                                    y_ps,
                                    lhsT=wo_sb[:, nn * 128 : (nn + 1) * 128],
                                    rhs=oT_sb[p],
                                    start=True,
                                    stop=True,
                                )
                                if nn % 2 == 0:
                                    nc.scalar.activation(
                                        y_sb[nn][:, p * 512 : (p + 1) * 512],
                                        y_ps, AF.Copy,
                                    )
                                else:
                                    nc.vector.tensor_copy(
                                        y_sb[nn][:, p * 512 : (p + 1) * 512], y_ps
                                    )
                                if p == 1:
                                    nc.sync.dma_start(
                                        out=yT_d[nn * 128 : (nn + 1) * 128, 0:1024],
                                        in_=y_sb[nn][:, 0:1024],
                                    )
                                elif p == TP - 1:
                                    nc.sync.dma_start(
                                        out=yT_d[nn * 128 : (nn + 1) * 128, 1024:2048],
                                        in_=y_sb[nn][:, 1024:2048],
                                    )


    nc.compile()
    return nc


def get_program():
    if "nc" not in _CACHE:
        _CACHE["nc"] = _build_program()
    return _CACHE["nc"]



def _wshuf(wT):
    """[1024 k, 128 n] -> [128 p, KC*128] with chunk kk at cols kk*128."""
    return np.ascontiguousarray(
        wT.reshape(KC, 128, DPC).transpose(1, 0, 2).reshape(128, KC * DPC)
    ).astype(BF16)

def make_in_maps(query, key, value, attention_mask, Wq, bq, Wk, bk, Wv, Wo):
    """Host-side sharding: per-core input dicts."""
    qT = np.ascontiguousarray(np.asarray(query, np.float32)[0].T).astype(BF16)
    kT = np.ascontiguousarray(np.asarray(key, np.float32)[0].T).astype(BF16)
    vT = np.ascontiguousarray(np.asarray(value, np.float32)[0].T).astype(BF16)
    maskT = np.ascontiguousarray(
        np.asarray(attention_mask, np.float32)[0, 0].T
    ).astype(FP8)

    in_maps = []
    for c in range(NCORES):
        ns = slice(c * DPC, (c + 1) * DPC)
        in_maps.append(
            {
                "qT": qT,
                "kT": kT,
                "vT": vT,
                "maskT": maskT,
                "wq": _wshuf(np.asarray(Wq, np.float32)[ns].T),
                "wk": _wshuf(np.asarray(Wk, np.float32)[ns].T),
                "wv": _wshuf(np.asarray(Wv, np.float32)[ns].T),
                "wo": np.ascontiguousarray(np.asarray(Wo, np.float32)[:, ns].T).astype(BF16),
                "bq": np.asarray(bq, np.float32)[None, ns].astype(BF16),
                "bk": np.asarray(bk, np.float32)[None, ns].astype(BF16),
                "ident": np.eye(128, dtype=BF16),
            }
        )
    return in_maps


def combine_outputs(results, Wv_bias, Wo, bo):
    """Sum per-core partial yT's (bf16 -> fp32), add host-folded biases."""
    acc = np.zeros((H, S), np.float32)
    for r in results:
        acc += r["yT"].astype(np.float32)
    bias = np.asarray(bo, np.float32) + np.asarray(Wv_bias, np.float32) @ np.asarray(
        Wo, np.float32
    ).T
    return (acc.T + bias[None, :]).astype(np.float32)[None]


def kernel(
    query,
    key,
    value,
    attention_mask,
    Wq,
    bq,
    Wk,
    bk,
    Wv,
    bv,
    Wo,
    bo,
    head,
    hidden_size,
):
    from concourse.bass_utils import run_bass_kernel_spmd

    nc = get_program()
    in_maps = make_in_maps(
        query, key, value, attention_mask, Wq, bq, Wk, bk, Wv, Wo
    )
    res = run_bass_kernel_spmd(nc, in_maps, list(range(NCORES)))
    return combine_outputs(res.results, bv, Wo, bo)

